# revision 1
# baseline (speedup 1.0000x reference)
"""MetaSR super-resolution Trainium2 kernel (bf16 pipelined version).

Structure exploited: out_h=out_w=256 with H=W=64 LR grid means the scale
factor is exactly 4, so the nearest-neighbor gather index is iy=oy//4,
ix=ox//4 and the per-query MLP input collapses to 16 distinct subpixel
phases [dy/4, dx/4, 0.25].  The whole model becomes:

  1. h    = relu(mlp_in @ w1 + b1)              [16, 256]
  2. predw = h @ w2 + b2                        [16, 576, 3]
  3. rgb[o, 4*iy+dy, 4*ix+dx] =
       sum_{c,ki,kj} feat[c, iy+ki-1, ix+kj-1] * predw[(dy,dx), c*9+ki*3+kj, o]
     i.e. a 3x3 conv with 64 in / 48 out channels + pixel shuffle.

Sharding: data-parallel over LR rows (8 rows per core, 10-row halo band),
weights replicated; steps 1+2 are recomputed on every core (tiny).

The conv contraction (K = 9 taps x 64 ch = 576) is chunked K=128 by pairing
taps.  Each core holds the zero-padded band twice in a 128-partition tile at
free-dim offsets that differ by the two taps' shift delta, so one K=128
matmul consumes two taps without materializing the unfolded tensor:
  band free index = r*66 + x  (66-wide zero-padded rows), tap (ki,kj) shift
  = ki*66+kj; taps are paired with shift deltas 1 or 64.

Changes vs the f32r baseline (28.5us -> ~21.7us):
  - All matmul operands are bf16: 1 cycle/row at any PE p-state, half the
    HBM traffic, and ~2x cheaper LDWEIGHTS for the 30 W-assembly loads
    (rel-err budget is 2e-2; bf16 everything lands ~5e-3).
  - w2 is split into 5 per-K-chunk DMAs and the band into 2, spread over
    both HWDGE rings in consumption order (PROC_ORDER follows arrivals),
    so the W-assembly/conv pipeline starts as soon as each block lands.
    The SP ring starts ~1.5us before the ACT ring; critical blocks ride SP.
  - w1/mlp_in ride a dense [3, 272] blob (3 partitions, not 128); b1 and
    the constant third mlp row are folded into the layer-1 matmul so the
    relu needs no bias blob on its critical path.
  - A run of dummy bf16 matmuls keeps the PE from idling cold before the
    first data lands (the PE clock ramps with activity; measured rate
    plateaus at ~0.83 ns/col on this part, so fills beyond that are waste).
  - Output is written as bf16 (host converts back to f32); the two
    output halves copy+DMA on separate rings to overlap the tail.

Measured phase structure (ticks ~0.83ns): ~7k of NEFF/runtime startup
(engine rendezvous + TENSOR_LOAD + preamble barriers) before any data
DMA, ~5.6k of input DMA overlapped with ~6.2k of PE work, ~2.5k output
tail, then ~7.5k of runtime-injected teardown (a per-semaphore clear of
the whole 256-entry file split across engines + final barriers).  The
startup/teardown ~12us is fixed for every NEFF on this stack; the
variable part here is within ~1.5us of its DMA/PE floor.  Run-to-run
variance is ~+/-1us.
"""

import os

import numpy as np

try:
    import concourse.bass as bass
except ImportError:  # fall back to the repo checkout
    import sys
    sys.path.insert(0, "/opt/trn_rl_repo")
    import concourse.bass as bass
import concourse.mybir as mybir
import concourse.tile as tile
from concourse import bacc
from concourse.bass_utils import run_bass_kernel_spmd

F32 = mybir.dt.float32
BF16 = mybir.dt.bfloat16
N_CORES = 8
ROWS_PER_CORE = 8          # LR rows per core
BAND_ROWS = ROWS_PER_CORE + 2
NPOS = ROWS_PER_CORE * 64  # 512 LR positions per core

# Tap order for K-chunking.  Taps t = ki*3+kj have band shift ki*66+kj:
#   t:      0   1   2   3    4    5    6    7    8
#   shift:  0   1   2   66   67   68   132  133  134
# chunk0: [t0; t1] band1 off 1 | chunk1: [t3; t2] band2 off 66
# chunk2: [t4; t5] band1 off 68 | chunk3: [t6; t7] band1 off 133
# chunk4: [t8] band2 off 134 (K=64)
TAP_ORDER = [0, 1, 3, 2, 4, 5, 6, 7, 8]
CHUNK_SPECS = [  # (band_tile_idx, rhs_offset, K)
    (0, 1, 128),
    (1, 66, 128),
    (0, 68, 128),
    (0, 133, 128),
    (1, 134, 64),
]
M_OFF = [0, 768, 1536, 2304, 3072]   # col offset of each m-block in w2b
COLS_W2 = 3072 + 384

OFF_BAND1 = 0
OFF_BAND2 = 661
COLS_BAND = 1385

N_WARMUP_MM = int(os.environ.get("METASR_WARMUP", "24"))
# dummy-matmul fill counts keeping the PE busy through DMA waits: the PE
# clock needs ~3us of gap-free work to reach full rate (idle resets it)
FILLS = [int(x) for x in os.environ.get("METASR_FILLS", "0,0,0,0,0").split(",")]

_CACHE = {}


def _build_program():
    """Build + compile the single-core Bass program (same for all cores)."""
    # Skip the all-engine barriers Bass.__init__ and the tile-context
    # entry emit before user code: they gate the first dma_start by ~1us,
    # this kernel never reads the const-ap region they fence (scalar
    # operands are immediates or b1b2 columns), and the tile framework
    # tracks all real data hazards.  The original barrier is restored
    # before the kernel body so the exit/teardown barriers (which order
    # the output DMAs before program end) are unaffected.
    orig_barrier = bass.Bass.all_engine_barrier
    bass.Bass.all_engine_barrier = lambda self, *a, **k: None
    nc = bacc.Bacc("TRN2", target_bir_lowering=False, debug=False)

    w1mlp_d = nc.dram_tensor("w1mlp", [3, 272], BF16, kind="ExternalInput")
    b1b2_d = nc.dram_tensor("b1b2", [128, 17], F32, kind="ExternalInput")
    w2b_d = nc.dram_tensor("w2b", [128, COLS_W2], BF16, kind="ExternalInput")
    band_d = nc.dram_tensor("band", [128, COLS_BAND], BF16, kind="ExternalInput")
    out48 = nc.dram_tensor("out48", [48, NPOS], BF16, kind="ExternalOutput")

    with tile.TileContext(nc) as tc:
        with (
            tc.tile_pool(name="blobs", bufs=1) as blobs,
            tc.tile_pool(name="work", bufs=1) as work,
            tc.tile_pool(name="wpool", bufs=5) as wpool,
            tc.tile_pool(name="ps_small", bufs=1, space="PSUM") as ps_small,
            tc.tile_pool(name="ps_w", bufs=5, space="PSUM") as ps_w,
            tc.tile_pool(name="ps_rgb", bufs=1, space="PSUM") as ps_rgb,
        ):
            # real barriers back on for the body + context teardown
            bass.Bass.all_engine_barrier = orig_barrier

            # ---- DMAs.  The SP ring starts ~1.5us earlier than the ACT
            # ring, so the critical-path blocks (smalls, m0, m1, band, m4)
            # ride SP; m2/m3 (needed mid-pipeline) ride ACT.
            b1b2 = blobs.tile([128, 17], F32, tag="b1b2")
            nc.sync.dma_start(b1b2[:, :], b1b2_d[:, :])
            w1mlp = blobs.tile([3, 272], BF16, tag="w1mlp")
            nc.sync.dma_start(w1mlp[:, :], w1mlp_d[:, :])

            w2b = blobs.tile([128, COLS_W2], BF16, tag="w2b")
            band = blobs.tile([128, COLS_BAND], BF16, tag="band")

            def w2_dma(eng, m):
                msz = 768 if m < 4 else 384
                eng.dma_start(
                    w2b[:, M_OFF[m]:M_OFF[m] + msz],
                    w2b_d[:, M_OFF[m]:M_OFF[m] + msz],
                )

            # SP: smalls, m0, band1, band2, m1 | ACT: m4, m2, m3.
            # m4 (98KB) rides ACT first so W4 is ready to fill the PE hole
            # while the bigger ACT blocks (m2, m3) are still in flight.
            w2_dma(nc.sync, 0)
            w2_dma(nc.scalar, 4)
            nc.sync.dma_start(band[:, 0:661], band_d[:, 0:661])
            w2_dma(nc.scalar, 2)
            nc.sync.dma_start(band[:, 661:COLS_BAND], band_d[:, 661:COLS_BAND])
            w2_dma(nc.scalar, 3)
            w2_dma(nc.sync, 1)

            band_tiles = [
                band[:, OFF_BAND1:OFF_BAND1 + 661],
                band[:, OFF_BAND2:OFF_BAND2 + 724],
            ]

            def w2_slice(m, o, hc, msize):
                base = M_OFF[m] + (o * 2 + hc) * msize
                return w2b[:, base:base + msize]

            # ---- PE warm-up: dummy zero matmuls into rgb_ps while DMAs
            # run.  conv chunk 0 below uses start=True, which resets the
            # PSUM accumulation, so these contribute nothing.
            rgb_ps = ps_rgb.tile([48, NPOS], F32, tag="rgb")
            warm = work.tile([128, 128], BF16, tag="warm")
            nc.vector.memset(warm[:, :], 0.0)
            for _ in range(N_WARMUP_MM):
                nc.tensor.matmul(
                    rgb_ps[:, 0:128], warm[:, 0:48], warm[:, :],
                    start=True, stop=True,
                )

            # ---- MLP layer 1: h_actT [256, 16] in two 128-chunks.
            # b1 and the constant mlp row (0.25) are folded host-side
            # into w1mlp (row 2 = b1 + 0.25*w1[2], mlp row 2 = 1), so
            # the bias is inside the matmul and relu needs no b1b2.
            h_sb = work.tile([128, 32], BF16, tag="hact")
            for hc in range(2):
                ph = ps_small.tile([128, 16], F32, tag="ph")
                nc.tensor.matmul(
                    ph[:, :], w1mlp[:, hc * 128:(hc + 1) * 128],
                    w1mlp[:, 256:272],
                    start=True, stop=True,
                )
                nc.vector.tensor_scalar(
                    h_sb[:, hc * 16:(hc + 1) * 16], ph[:, :],
                    0.0, 0.0,
                    mybir.AluOpType.max, mybir.AluOpType.max,
                )

            def fill(n):
                # dummy matmuls into rgb_ps (the first real conv uses
                # start=True, resetting the accumulation) to keep the
                # PE clock ramped while waiting on DMA arrivals
                for _ in range(n):
                    nc.tensor.matmul(
                        rgb_ps[:, 0:128], warm[:, 0:48], warm[:, :],
                        start=True, stop=True,
                    )

            # ---- W assembly (MLP layer 2) and convs in an explicit
            # emission order chosen to match DMA arrivals: W4 fills the
            # wait for m2/m3; band1-only convs (c2, c3) run before the
            # band2-dependent c4; m1's W1+c1 close the chain.
            EMIT = [("W", 0), ("c", 0), ("W", 4), ("W", 2), ("c", 2),
                    ("W", 3), ("c", 3), ("c", 4), ("W", 1), ("c", 1)]
            conv_ms = [m for kind, m in EMIT if kind == "c"]

            def conv(m):
                bidx, roff, K = CHUNK_SPECS[m]
                bt = band_tiles[bidx]
                rhs = bt[0:K, roff:roff + 8 * 66].rearrange(
                    "p (r c) -> p r c", c=66
                )[:, :, 0:64]
                nc.tensor.matmul(
                    rgb_ps[:, :], w_sbs[m][:K, :], rhs,
                    start=(m == conv_ms[0]), stop=(m == conv_ms[-1]),
                )

            fill(FILLS[0])
            w_sbs = {}
            for kind, m in EMIT:
                if kind == "c":
                    conv(m)
                    continue
                bidx, roff, K = CHUNK_SPECS[m]
                msize = K
                w_sb = wpool.tile([128, 48], BF16, tag="W")
                w_sbs[m] = w_sb
                for o in range(3):
                    pw = ps_w.tile([128, 16], F32, tag="pw")
                    for hc in range(2):
                        nc.tensor.matmul(
                            pw[:msize, :],
                            w2_slice(m, o, hc, msize),
                            h_sb[:, hc * 16:(hc + 1) * 16],
                            start=(hc == 0), stop=(hc == 1),
                        )
                    nc.vector.tensor_scalar_add(
                        w_sb[:msize, o * 16:(o + 1) * 16], pw[:msize, :],
                        b1b2[:msize, 2 + o * 5 + m:3 + o * 5 + m],
                    )

            # ---- write out in bf16 (host converts back to f32); half 0's
            # copy+DMA overlap half 1's copy.  (Cross-engine casts of the
            # same PSUM tile get serialized by the framework, so both
            # casts stay on the DVE.)
            out_sb = work.tile([48, NPOS], BF16, tag="out")
            for half in range(2):
                cs = half * (NPOS // 2)
                nc.vector.tensor_copy(
                    out_sb[:, cs:cs + NPOS // 2], rgb_ps[:, cs:cs + NPOS // 2]
                )
                eng = nc.scalar if half == 0 else nc.sync
                eng.dma_start(
                    out48[:, cs:cs + NPOS // 2], out_sb[:, cs:cs + NPOS // 2]
                )

    nc.compile()
    return nc


def _to_bf16_bits(x):
    """fp32 -> bf16 (round-to-nearest-even) as a uint16-safe numpy view."""
    import ml_dtypes
    return x.astype(ml_dtypes.bfloat16)


def _host_prep(feat, w1, b1, w2, b2):
    """Pack shared blobs + per-core band blobs."""
    import ml_dtypes
    feat = np.ascontiguousarray(np.asarray(feat, dtype=np.float32))[0]  # [64,64,64]
    w1 = np.asarray(w1, dtype=np.float32)
    b1 = np.asarray(b1, dtype=np.float32)
    w2 = np.asarray(w2, dtype=np.float32)
    b2 = np.asarray(b2, dtype=np.float32)

    dydx = np.arange(16)
    mlpin = np.stack(
        [dydx // 4 / 4.0, dydx % 4 / 4.0, np.full(16, 0.25)], axis=0
    ).astype(np.float32)  # [3, 16]

    # fold b1 + 0.25*w1[2] into row 2 of the w1 blob (mlp row 2 becomes
    # ones), so MLP layer 1's bias rides inside the matmul
    w1mlp = np.zeros((3, 272), dtype=np.float32)
    w1mlp[0:2, 0:256] = w1[0:2]
    w1mlp[2, 0:256] = b1 + 0.25 * w1[2]
    w1mlp[0:2, 256:272] = mlpin[0:2]
    w1mlp[2, 256:272] = 1.0
    w1mlp = _to_bf16_bits(w1mlp)

    # tap-major permutations of w2/b2
    w2r = w2.reshape(256, 64, 9, 3)  # [h, c, t, o]
    w2p = np.empty((3, 256, 576), dtype=np.float32)
    b2r = b2.reshape(64, 9, 3)       # [c, t, o]
    b2p = np.empty((3, 576), dtype=np.float32)
    for blk, t in enumerate(TAP_ORDER):
        w2p[:, :, blk * 64:(blk + 1) * 64] = w2r[:, :, t, :].transpose(2, 0, 1)
        b2p[:, blk * 64:(blk + 1) * 64] = b2r[:, t, :].T

    b1b2 = np.zeros((128, 17), dtype=np.float32)
    b1b2[:, 0] = b1[0:128]
    b1b2[:, 1] = b1[128:256]
    for o in range(3):
        for m in range(5):
            msize = 128 if m < 4 else 64
            b1b2[:msize, 2 + o * 5 + m] = b2p[o, 128 * m:128 * m + msize]

    w2b = np.empty((128, COLS_W2), dtype=ml_dtypes.bfloat16)
    w2p16 = _to_bf16_bits(w2p)
    for m in range(5):
        msize = 128 if m < 4 else 64
        for o in range(3):
            for hc in range(2):
                base = M_OFF[m] + (o * 2 + hc) * msize
                w2b[:, base:base + msize] = \
                    w2p16[o, hc * 128:(hc + 1) * 128, 128 * m:128 * m + msize]

    featp = np.zeros((64, 66, 66), dtype=np.float32)
    featp[:, 1:65, 1:65] = feat
    featp = _to_bf16_bits(featp)

    blobs_band = []
    for core in range(N_CORES):
        r0 = core * ROWS_PER_CORE
        band = featp[:, r0:r0 + BAND_ROWS, :].reshape(64, BAND_ROWS * 66)
        bb = np.zeros((128, COLS_BAND), dtype=ml_dtypes.bfloat16)
        bb[0:64, OFF_BAND1 + 1:OFF_BAND1 + 661] = band
        bb[64:128, OFF_BAND1 + 0:OFF_BAND1 + 660] = band
        bb[0:64, OFF_BAND2 + 0:OFF_BAND2 + 660] = band
        bb[64:128, OFF_BAND2 + 64:OFF_BAND2 + 724] = band
        blobs_band.append(bb)
    return w1mlp, b1b2, w2b, blobs_band


def _assemble(per_core_out48):
    """[8 x [48, 512]] -> [1, 3, 256, 256]."""
    full = np.stack([np.asarray(o, dtype=np.float32) for o in per_core_out48])
    full = full.reshape(8, 3, 4, 4, 8, 64)               # [core, o, dy, dx, r, x]
    rgb = full.transpose(1, 0, 4, 2, 5, 3).reshape(3, 256, 256)
    return np.ascontiguousarray(rgb)[None]


def get_program():
    if "nc" not in _CACHE:
        _CACHE["nc"] = _build_program()
    return _CACHE["nc"]


def run(feat, w1, b1, w2, b2, out_h, out_w, trace=False, **spmd_kwargs):
    assert int(out_h) == 256 and int(out_w) == 256
    nc = get_program()
    w1mlp, b1b2, w2b, blobs_band = _host_prep(feat, w1, b1, w2, b2)
    in_maps = [
        {"w1mlp": w1mlp, "b1b2": b1b2, "w2b": w2b, "band": blobs_band[core]}
        for core in range(N_CORES)
    ]
    res = run_bass_kernel_spmd(
        nc, in_maps, core_ids=list(range(N_CORES)), trace=trace, **spmd_kwargs
    )
    out = _assemble([res.results[core]["out48"] for core in range(N_CORES)])
    return out, res


def kernel(feat, w1, b1, w2, b2, out_h, out_w):
    out, _ = run(feat, w1, b1, w2, b2, out_h, out_w, trace=False)
    return out

